# revision 55
# baseline (speedup 1.0000x reference)
"""Multi-head attention (b=4, n=2048, dim=1024, 16 heads x 64) on 8 Trainium2
NeuronCores.

Sharding: data-parallel over batch (4) x tensor-parallel over head-groups (2).
Each core gets one batch element and 8 heads: it computes its slice of the QKV
projection, full attention for its heads, and a partial output projection.
The host sums the two head-group partials per batch element and adds b_out.

Per-core pipeline (fp32 data; matmul-feeding tiles float32r):
  A:  per 512-wide n-chunk: PE-transpose x tiles into xT chunk tiles (SBUF),
      then qT = Wq^T x^T (staged to DRAM, streamed back in B), kT = Wk^T x^T
      (SBUF-resident, [inner, n] in 128-row strips) and v = x Wv (natural
      [n, inner], augmented with a ones column per head so the PV matmul also
      emits the softmax denominator).
  B:  i-blocks (ib) outer, heads inner: S^T j-tiles = matmul(lhsT=k^T_h
      j-block, rhs=q^T_h i-block) ([j, i] scores); exp on ScalarE
      (1/sqrt(dh) folded into the activation scale); PV matmul accumulates
      O_aug^T = v_aug^T @ P^T in PSUM half-blocks ([dh+1, 512]; last row =
      denominator). Tail: reciprocal of the denominator row, broadcast across
      partitions on GPSIMD, multiply -> normalized O^T strip. The PV matmuls
      trail S/exp by one step in a FIFO that carries across head boundaries,
      so ScalarE (the phase-B bottleneck) never runs dry.
  C:  y = O @ w_out via lhsT = O^T strips, emitted in single-PSUM-group
      slices woven through the NEXT i-block's heads (fills PE slack).
  A/B overlap: emission of B is interleaved into A's chunk emission as soon
  as the chunks a step needs are complete, so ScalarE starts exp work while
  the PE is still on the QKV projection.
"""

from contextlib import ExitStack

import numpy as np

import concourse.bass as bass
import concourse.mybir as mybir
import concourse.tile as tile
from concourse import bacc, bass_utils
from concourse.masks import make_identity

F32 = mybir.dt.float32
AF = mybir.ActivationFunctionType

# Full-problem constants (hardcoded per the harness contract).
B_FULL, N_FULL, DIM_FULL = 4, 2048, 1024
HEADS_FULL, DH = 16, 64
N_CORES = 8
GROUPS = 2                       # head-group (tensor-parallel) factor
HPC = HEADS_FULL // GROUPS       # heads per core = 8
INNER_PC = HPC * DH              # per-core inner dim = 512

# Matmul compute dtype: float32r streams 1 row/cycle (vs 4 for float32) at
# slightly reduced precision. All tiles feeding matmuls carry this dtype
# (producers round into it); numpy float32 maps onto it unchanged.
MM_DT = mybir.dt.float32r


def ts(i, size):
    return slice(i * size, (i + 1) * size)


def emit_core_kernel(nc, tc, x, wqkv, wout, y, *, n, dim, hpc, dh,
                     mm_dt=MM_DT, ib=1024, bcast="gpsimd", overlap=True):
    inner = hpc * dh
    KC = dim // 128          # contraction chunks for the qkv projection
    S = inner // 128         # 128-row strips of the per-core inner dim
    JT = n // 128            # key/value j-tiles
    NB = n // 512            # 512-wide n-chunks in phase A
    ib = min(ib, n)
    assert n % 512 == 0 and dim % 128 == 0 and inner % 128 == 0
    assert ib % 512 == 0 and n % ib == 0
    scale = float(1.0 / np.sqrt(dh))
    MD = mm_dt
    fc = min(512, dim)
    n_ibx = n // ib
    itpb = ib // 128                 # i-tiles per i-block
    state = {"chunk_done": -1}

    stack = ExitStack()
    with stack:
        const_pool = stack.enter_context(tc.tile_pool(name="const", bufs=1))
        persist = stack.enter_context(tc.tile_pool(name="persist", bufs=1))
        dram_pool = stack.enter_context(
            tc.tile_pool(name="dram", bufs=1, space="DRAM"))

        qt_dram = dram_pool.tile([S, 128, n], MD, name="qt_dram")

        # Constants are embedded in the NEFF and DMA'd in (no gpsimd on the
        # startup critical path). Anything that feeds a matmul is rounded
        # into mm_dt via DVE copies.
        ident = const_pool.tile([128, 128], F32, name="ident")
        nc.sync.dma_start(
            ident, nc.inline_tensor(np.eye(128, dtype=np.float32),
                                    name="ident_const").ap())
        oneshc = const_pool.tile([128, hpc], F32, name="oneshc")
        nc.sync.dma_start(
            oneshc, nc.inline_tensor(np.ones((128, hpc), np.float32),
                                     name="ones_const").ap())
        if bcast == "matmul":
            ones_f32 = const_pool.tile([1, dh], F32, name="ones_f32")
            nc.gpsimd.memset(ones_f32, 1.0)
            ones_sb = const_pool.tile([1, dh], MD, name="ones_sb")
            nc.vector.tensor_copy(ones_sb, ones_f32)

        # Persistent SBUF tensors: kT strips, v_aug tiles, oT strip 0 (the
        # strip the first two heads write during the A/B overlap; strips 1+
        # are allocated after phase A's pools release).
        kT = []
        for s in range(S):
            kT.append(persist.tile([128, n], MD, name="kTs", tag=f"kT{s}"))
        v_sb = []
        for jt in range(JT):
            vt = persist.tile([128, hpc * (dh + 1)], MD, name="vts",
                              tag=f"v{jt}")
            v_sb.append(vt)
            nc.vector.tensor_copy(
                vt.rearrange("p (h c) -> p h c", c=dh + 1)[:, :, dh:dh + 1],
                oneshc.rearrange("p (h c) -> p h c", c=1))
        oT = []                      # strips allocated after phase A

        # ---- phase A pools ----
        actx = ExitStack()
        w_pool = actx.enter_context(tc.tile_pool(name="a_w", bufs=1))
        xin_pool = actx.enter_context(tc.tile_pool(name="a_xin", bufs=4))
        xts_pool = actx.enter_context(tc.tile_pool(name="a_xts", bufs=2))
        qstage_pool = actx.enter_context(
            tc.tile_pool(name="a_qstage", bufs=2))
        psT_pool = actx.enter_context(
            tc.tile_pool(name="a_psT", bufs=2, space="PSUM"))
        psA_pool = actx.enter_context(
            tc.tile_pool(name="a_ps", bufs=3, space="PSUM"))

        def a_emit():
            # First x tiles are on the critical path; their DMAs go first.
            first_x = []
            for j2 in range(4):
                x_in = xin_pool.tile([128, dim], F32, name="x_in")
                nc.sync.dma_start(x_in, x[ts(j2, 128), :])
                first_x.append(x_in)
            w_sb = []
            for kc in range(KC):
                wt = w_pool.tile([128, 3 * inner], MD, name="wt",
                                 tag=f"w{kc}")
                nc.sync.dma_start(wt, wqkv[ts(kc, 128), :])
                w_sb.append(wt)

            for nb in range(NB):
                xts = [xts_pool.tile([128, 512], MD, name="xts",
                                     tag=f"xts{kc}") for kc in range(KC)]
                for j2 in range(4):
                    it = nb * 4 + j2
                    if nb == 0:
                        x_in = first_x[j2]
                    else:
                        x_in = xin_pool.tile([128, dim], F32, name="x_in")
                        nc.sync.dma_start(x_in, x[ts(it, 128), :])
                    for kc in range(KC):
                        pt = psT_pool.tile([128, 128], F32, name="pt")
                        nc.tensor.transpose(pt, x_in[:, ts(kc, 128)], ident)
                        nc.vector.tensor_copy(xts[kc][:, ts(j2, 128)], pt)
                    yield None
                # qT (to DRAM) / kT (SBUF) strips over this 512-wide chunk
                for which in (0, 1):
                    for s in range(S):
                        ps = psA_pool.tile([128, 512], F32, name="psA")
                        base = which * inner + s * 128
                        for kc in range(KC):
                            nc.tensor.matmul(
                                ps, w_sb[kc][:, base:base + 128], xts[kc],
                                start=(kc == 0), stop=(kc == KC - 1))
                        if which == 1:
                            nc.vector.tensor_copy(kT[s][:, ts(nb, 512)], ps)
                        else:
                            qs = qstage_pool.tile([128, 512], MD, name="qs")
                            nc.vector.tensor_copy(qs, ps)
                            nc.sync.dma_start(qt_dram[s, :, ts(nb, 512)], qs)
                        yield None
                # v natural: 4 row-tiles of 128 within this chunk
                for j2 in range(4):
                    it = nb * 4 + j2
                    ps = psA_pool.tile([128, inner], F32, name="psAv",
                                       tag="psAv")
                    for kc in range(KC):
                        nc.tensor.matmul(
                            ps, xts[kc][:, ts(j2, 128)],
                            w_sb[kc][:, 2 * inner:3 * inner],
                            start=(kc == 0), stop=(kc == KC - 1))
                    nc.vector.tensor_copy(
                        v_sb[it].rearrange(
                            "p (h c) -> p h c", c=dh + 1)[:, :, 0:dh],
                        ps.rearrange("p (h c) -> p h c", c=dh))
                    yield None
                state["chunk_done"] = nb

        # ---- phase B/C emission (single generator; yields the A-chunk
        #      index the NEXT step needs before emitting it) ----
        wout_sb = []                  # filled after phase A pools release
        ysb_open = {}
        pend = []                     # (po, pexp, jt, h, ibx)
        proj_due = []
        qst_tiles = {}
        seq = [(bx, hh) for bx in range(n_ibx) for hh in range(hpc)]

        def qst_req(bx):
            return ((bx + 1) * ib - 1) // 512

        def load_qst(i):
            if i < len(seq) and i not in qst_tiles:
                bx, hh = seq[i]
                if qst_req(bx) > state["chunk_done"]:
                    return
                s2, r2 = divmod(hh * dh, 128)
                t = qst_pool.tile([128, ib], MD, name="qst")
                nc.sync.dma_start(
                    t[r2:r2 + dh, :], qt_dram[s2, r2:r2 + dh, ts(bx, ib)])
                qst_tiles[i] = t

        def emit_tail(po_c, h, ibx, c):
            # normalize rows 0..dh-1 of one half-block by its denominator
            s_, r_ = divmod(h * dh, 128)
            recip_f = tail_pool.tile([1, 512], F32, name="recip_f")
            nc.vector.reciprocal(recip_f, po_c[dh:dh + 1, :])
            bc = tail_pool.tile([dh, 512], F32, name="bc")
            if bcast == "gpsimd":
                nc.gpsimd.partition_broadcast(bc, recip_f)
            else:
                recip = tail_pool.tile([1, 512], MD, name="recip")
                nc.vector.tensor_copy(recip, recip_f)
                pb = psB_pool.tile([dh, 512], F32, name="pb")
                nc.tensor.matmul(pb, ones_sb, recip, start=True, stop=True)
                nc.vector.tensor_copy(bc, pb)
            off = ibx * ib + c * 512
            nc.vector.tensor_mul(
                oT[s_][r_:r_ + dh, off:off + 512], po_c[0:dh, :], bc)

        def emit_proj_group(it, c):
            # one PSUM-group slice of the projection for i-tile `it`
            if c == 0:
                ysb_open[it] = y_pool.tile([128, dim], F32, name="ysb")
            ysb = ysb_open[it]
            ps = psC_pool.tile([128, fc], F32, name="psC")
            for t in range(S):
                nc.tensor.matmul(
                    ps, oT[t][:, ts(it, 128)], wout_sb[t][:, ts(c, fc)],
                    start=(t == 0), stop=(t == S - 1))
            nc.vector.tensor_copy(ysb[:, ts(c, fc)], ps)
            if c == dim // fc - 1:
                nc.sync.dma_start(y[ts(it, 128), :], ysb)
                del ysb_open[it]

        def pop_pend():
            po, pexp, jt, h, ibx = pend.pop(0)
            vcol = slice(h * (dh + 1), (h + 1) * (dh + 1))
            for c in range(ib // 512):
                nc.tensor.matmul(
                    po[c], v_sb[jt][:, vcol], pexp[:, ts(c, 512)],
                    start=(jt == 0), stop=(jt == JT - 1))
            if jt == JT - 1:
                for c in range(ib // 512):
                    emit_tail(po[c], h, ibx, c)

        def b_emit():
            nonlocal proj_due
            for ibx in range(n_ibx):
                for h in range(hpc):
                    gi = ibx * hpc + h
                    yield qst_req(ibx)
                    load_qst(gi)
                    load_qst(gi + 1)
                    s_, r_ = divmod(h * dh, 128)
                    kTh = kT[s_][r_:r_ + dh, :]
                    qTh = qst_tiles.pop(gi)[r_:r_ + dh, :]
                    po = [psO_pool.tile([dh + 1, 512], F32, name="po")
                          for _ in range(ib // 512)]
                    spread = max(1, JT // max(1, -(-len(proj_due) // hpc) + 1))
                    for jt in range(JT):
                        if jt:
                            yield max(qst_req(ibx), jt // 4)
                        psS = psS_pool.tile([128, ib], F32, name="psS")
                        for c in range(ib // 512):
                            nc.tensor.matmul(
                                psS[:, ts(c, 512)], kTh[:, ts(jt, 128)],
                                qTh[:, ts(c, 512)],
                                start=True, stop=True)
                        pexp = pexp_pool.tile([128, ib], MD, name="pexp")
                        nc.scalar.activation(pexp, psS, AF.Exp, scale=scale)
                        pend.append((po, pexp, jt, h, ibx))
                        while len(pend) > 1:
                            pop_pend()
                        if (proj_due and jt % spread == spread - 1
                                and jt < JT - 1):
                            emit_proj_group(*proj_due.pop(0))
                while proj_due:   # leftovers from the previous block
                    emit_proj_group(*proj_due.pop(0))
                proj_due = [(it, c)
                            for it in range(ibx * itpb, (ibx + 1) * itpb)
                            for c in range(dim // fc)]
                if ibx == n_ibx - 1:
                    while pend:
                        pop_pend()
                    for it, c in proj_due:
                        emit_proj_group(it, c)
                    proj_due = []

        # ---- drive phase A, then phase B/C ----
        for _ in a_emit():
            pass
        actx.close()      # release phase A pools

        # phase B/C pools live in the space freed by phase A
        with (
            tc.tile_pool(name="b_psS", bufs=2, space="PSUM") as psS_pool_,
            tc.tile_pool(name="b_psO", bufs=3, space="PSUM") as psO_pool_,
            tc.tile_pool(name="c_ps", bufs=1, space="PSUM") as psC_pool,
            tc.tile_pool(name="b_pexp", bufs=3) as pexp_pool,
            tc.tile_pool(name="b_qst", bufs=4) as qst_pool,
            tc.tile_pool(name="b_tail", bufs=2) as tail_pool,
            tc.tile_pool(name="c_w", bufs=1) as wout_pool,
            tc.tile_pool(name="c_y", bufs=3) as y_pool,
        ):
            psS_pool, psO_pool = psS_pool_, psO_pool_
            if bcast == "matmul":
                psB_pool = stack.enter_context(
                    tc.tile_pool(name="b_psB", bufs=1, space="PSUM"))
            for s in range(S):
                oT.append(persist.tile([128, n], MD, name="oTs",
                                       tag=f"oT{s}"))
            for t in range(S):
                wo = wout_pool.tile([128, dim], MD, name="wo", tag=f"wo{t}")
                nc.sync.dma_start(wo, wout[ts(t, 128), :])
                wout_sb.append(wo)
            for _ in b_emit():
                pass


_BUILD_CACHE = {}


def build_nc(n=N_FULL, dim=DIM_FULL, hpc=HPC, dh=DH, mm_dt=MM_DT, ib=1024,
             bcast="gpsimd", overlap=True):
    key = (n, dim, hpc, dh, str(mm_dt), ib, bcast, overlap)
    if key in _BUILD_CACHE:
        return _BUILD_CACHE[key]
    inner = hpc * dh
    nc = bacc.Bacc("TRN2", target_bir_lowering=False, debug=False)
    x = nc.dram_tensor("x", [n, dim], F32, kind="ExternalInput").ap()
    wqkv = nc.dram_tensor("w_qkv", [dim, 3 * inner], mm_dt,
                          kind="ExternalInput").ap()
    wout = nc.dram_tensor("w_out", [inner, dim], mm_dt,
                          kind="ExternalInput").ap()
    y = nc.dram_tensor("y", [n, dim], F32, kind="ExternalOutput").ap()
    with tile.TileContext(nc) as tc:
        with nc.allow_low_precision(
                reason="float32r is 4-byte; PSUM accumulation stays fp32"):
            emit_core_kernel(nc, tc, x, wqkv, wout, y, n=n, dim=dim, hpc=hpc,
                             dh=dh, mm_dt=mm_dt, ib=ib, bcast=bcast,
                             overlap=overlap)
    nc.compile()
    _BUILD_CACHE[key] = nc
    return nc


def make_in_maps(x, w_qkv, w_out):
    """Shard full inputs into the 8 per-core input maps."""
    x = np.asarray(x, dtype=np.float32)
    w_qkv = np.asarray(w_qkv, dtype=np.float32)
    w_out = np.asarray(w_out, dtype=np.float32)
    qk_off = HEADS_FULL * DH          # 1024: start of K block in w_qkv
    in_maps = []
    for c in range(N_CORES):
        b, g = divmod(c, GROUPS)
        cols = ts(g, INNER_PC)
        wq = w_qkv[:, cols]
        wk = w_qkv[:, qk_off + g * INNER_PC: qk_off + (g + 1) * INNER_PC]
        wv = w_qkv[:, 2 * qk_off + g * INNER_PC: 2 * qk_off + (g + 1) * INNER_PC]
        in_maps.append({
            "x": np.ascontiguousarray(x[b]),
            "w_qkv": np.ascontiguousarray(np.concatenate([wq, wk, wv], axis=1)),
            "w_out": np.ascontiguousarray(w_out[cols, :]),
        })
    return in_maps


def kernel(x, w_qkv, w_out, b_out, trace=False):
    b_out = np.asarray(b_out, dtype=np.float32)
    nc = build_nc()
    in_maps = make_in_maps(x, w_qkv, w_out)
    res = bass_utils.run_bass_kernel_spmd(
        nc, in_maps, core_ids=list(range(N_CORES)), trace=trace)
    ys = [r["y"] for r in res.results]
    out = np.empty((B_FULL, N_FULL, DIM_FULL), dtype=np.float32)
    for b in range(B_FULL):
        out[b] = ys[GROUPS * b] + ys[GROUPS * b + 1] + b_out[None, :]
    if trace:
        kernel.last_result = res
    return out


# revision 58
# speedup vs baseline: 16161.5147x; 16161.5147x over previous
"""Multi-head attention (b=4, n=2048, dim=1024, 16 heads x 64) on 8 Trainium2
NeuronCores.

Sharding: data-parallel over batch (4) x tensor-parallel over head-groups (2).
Each core gets one batch element and 8 heads: it computes its slice of the QKV
projection, full attention for its heads, and a partial output projection.
The host sums the two head-group partials per batch element and adds b_out.

Per-core pipeline (fp32 data; matmul-feeding tiles float32r):
  A:  per 512-wide n-chunk: PE-transpose x tiles into xT chunk tiles (SBUF),
      then qT = Wq^T x^T (staged to DRAM, streamed back in B), kT = Wk^T x^T
      (SBUF-resident, [inner, n] in 128-row strips) and v = x Wv (natural
      [n, inner], augmented with a ones column per head so the PV matmul also
      emits the softmax denominator).
  B:  i-blocks (ib) outer, heads inner: S^T j-tiles = matmul(lhsT=k^T_h
      j-block, rhs=q^T_h i-block) ([j, i] scores); exp on ScalarE
      (1/sqrt(dh) folded into the activation scale); PV matmul accumulates
      O_aug^T = v_aug^T @ P^T in PSUM half-blocks ([dh+1, 512]; last row =
      denominator). Tail: reciprocal of the denominator row, broadcast across
      partitions on GPSIMD, multiply -> normalized O^T strip. The PV matmuls
      trail S/exp by one step in a FIFO that carries across head boundaries,
      so ScalarE (the phase-B bottleneck) never runs dry.
  C:  y = O @ w_out via lhsT = O^T strips, emitted in single-PSUM-group
      slices woven through the NEXT i-block's heads (fills PE slack).
  A/B overlap: emission of B is interleaved into A's chunk emission as soon
  as the chunks a step needs are complete, so ScalarE starts exp work while
  the PE is still on the QKV projection.
"""

from contextlib import ExitStack

import numpy as np

import concourse.mybir as mybir
import concourse.tile as tile
from concourse import bacc, bass_utils

F32 = mybir.dt.float32
AF = mybir.ActivationFunctionType

# Full-problem constants (hardcoded per the harness contract).
B_FULL, N_FULL, DIM_FULL = 4, 2048, 1024
HEADS_FULL, DH = 16, 64
N_CORES = 8
GROUPS = 2                       # head-group (tensor-parallel) factor
HPC = HEADS_FULL // GROUPS       # heads per core = 8
INNER_PC = HPC * DH              # per-core inner dim = 512

# Matmul compute dtype: float32r streams 1 row/cycle (vs 4 for float32) at
# slightly reduced precision. All tiles feeding matmuls carry this dtype
# (producers round into it); numpy float32 maps onto it unchanged.
MM_DT = mybir.dt.float32r


def ts(i, size):
    return slice(i * size, (i + 1) * size)


def emit_core_kernel(nc, tc, x, wqkv, wout, y, *, n, dim, hpc, dh,
                     mm_dt=MM_DT, ib=1024, bcast="gpsimd", overlap=True):
    inner = hpc * dh
    KC = dim // 128          # contraction chunks for the qkv projection
    S = inner // 128         # 128-row strips of the per-core inner dim
    JT = n // 128            # key/value j-tiles
    NB = n // 512            # 512-wide n-chunks in phase A
    ib = min(ib, n)
    assert n % 512 == 0 and dim % 128 == 0 and inner % 128 == 0
    assert ib % 512 == 0 and n % ib == 0
    scale = float(1.0 / np.sqrt(dh))
    MD = mm_dt
    fc = min(512, dim)
    n_ibx = n // ib
    itpb = ib // 128                 # i-tiles per i-block
    state = {"chunk_done": -1}

    stack = ExitStack()
    with stack:
        const_pool = stack.enter_context(tc.tile_pool(name="const", bufs=1))
        persist = stack.enter_context(tc.tile_pool(name="persist", bufs=1))
        dram_pool = stack.enter_context(
            tc.tile_pool(name="dram", bufs=1, space="DRAM"))

        qt_dram = dram_pool.tile([S, 128, n], MD, name="qt_dram")

        # Constants are embedded in the NEFF and DMA'd in (no gpsimd on the
        # startup critical path). Anything that feeds a matmul is rounded
        # into mm_dt via DVE copies.
        ident = const_pool.tile([128, 128], F32, name="ident")
        nc.sync.dma_start(
            ident, nc.inline_tensor(np.eye(128, dtype=np.float32),
                                    name="ident_const").ap())
        oneshc = const_pool.tile([128, hpc], F32, name="oneshc")
        nc.sync.dma_start(
            oneshc, nc.inline_tensor(np.ones((128, hpc), np.float32),
                                     name="ones_const").ap())
        if bcast == "matmul":
            ones_f32 = const_pool.tile([1, dh], F32, name="ones_f32")
            nc.gpsimd.memset(ones_f32, 1.0)
            ones_sb = const_pool.tile([1, dh], MD, name="ones_sb")
            nc.vector.tensor_copy(ones_sb, ones_f32)

        # Persistent SBUF tensors: kT strips, v_aug tiles, oT strip 0 (the
        # strip the first two heads write during the A/B overlap; strips 1+
        # are allocated after phase A's pools release).
        kT = []
        for s in range(S):
            kT.append(persist.tile([128, n], MD, name="kTs", tag=f"kT{s}"))
        v_sb = []
        for jt in range(JT):
            vt = persist.tile([128, hpc * (dh + 1)], MD, name="vts",
                              tag=f"v{jt}")
            v_sb.append(vt)
            nc.vector.tensor_copy(
                vt.rearrange("p (h c) -> p h c", c=dh + 1)[:, :, dh:dh + 1],
                oneshc.rearrange("p (h c) -> p h c", c=1))
        oT = []                      # strips allocated after phase A

        # ---- phase A pools ----
        actx = ExitStack()
        w_pool = actx.enter_context(tc.tile_pool(name="a_w", bufs=1))
        xin_pool = actx.enter_context(tc.tile_pool(name="a_xin", bufs=4))
        xts_pool = actx.enter_context(tc.tile_pool(name="a_xts", bufs=2))
        qstage_pool = actx.enter_context(
            tc.tile_pool(name="a_qstage", bufs=2))
        psT_pool = actx.enter_context(
            tc.tile_pool(name="a_psT", bufs=2, space="PSUM"))
        psA_pool = actx.enter_context(
            tc.tile_pool(name="a_ps", bufs=3, space="PSUM"))

        def emit_a():
            # First x tiles are on the critical path; their DMAs go first.
            first_x = []
            for j2 in range(4):
                x_in = xin_pool.tile([128, dim], F32, name="x_in")
                nc.sync.dma_start(x_in, x[ts(j2, 128), :])
                first_x.append(x_in)
            w_sb = []
            for kc in range(KC):
                wt = w_pool.tile([128, 3 * inner], MD, name="wt",
                                 tag=f"w{kc}")
                nc.sync.dma_start(wt, wqkv[ts(kc, 128), :])
                w_sb.append(wt)

            def transpose_ops(nb, xts):
                # lazily emitted (x_in DMA, transpose, copy) for one chunk
                for j2 in range(4):
                    it = nb * 4 + j2
                    if nb == 0:
                        x_in = first_x[j2]
                    else:
                        x_in = xin_pool.tile([128, dim], F32, name="x_in")
                        nc.sync.dma_start(x_in, x[ts(it, 128), :])
                    for kc in range(KC):
                        pt = psT_pool.tile([128, 128], F32, name="pt")
                        nc.tensor.transpose(pt, x_in[:, ts(kc, 128)], ident)
                        nc.vector.tensor_copy(xts[kc][:, ts(j2, 128)], pt)
                        yield None

            def alloc_xts():
                return [xts_pool.tile([128, 512], MD, name="xts",
                                      tag=f"xts{kc}") for kc in range(KC)]

            # transposes run one chunk ahead, woven between matmul groups
            # so the PE is never paced by the DVE copies draining PSUM.
            xts = alloc_xts()
            for _ in transpose_ops(0, xts):
                pass
            for nb in range(NB):
                if nb + 1 < NB:
                    xts_next = alloc_xts()
                    tq = transpose_ops(nb + 1, xts_next)
                else:
                    xts_next, tq = None, iter(())

                def weave(k=3):
                    for _ in range(k):
                        next(tq, None)

                # qT (to DRAM) / kT (SBUF) strips over this 512-wide chunk
                for which in (0, 1):
                    for s in range(S):
                        ps = psA_pool.tile([128, 512], F32, name="psA")
                        base = which * inner + s * 128
                        for kc in range(KC):
                            nc.tensor.matmul(
                                ps, w_sb[kc][:, base:base + 128], xts[kc],
                                start=(kc == 0), stop=(kc == KC - 1))
                        if which == 1:
                            nc.vector.tensor_copy(kT[s][:, ts(nb, 512)], ps)
                        else:
                            qs = qstage_pool.tile([128, 512], MD, name="qs")
                            nc.vector.tensor_copy(qs, ps)
                            nc.sync.dma_start(qt_dram[s, :, ts(nb, 512)], qs)
                        weave()
                # v natural: 4 row-tiles of 128 within this chunk
                for j2 in range(4):
                    it = nb * 4 + j2
                    ps = psA_pool.tile([128, inner], F32, name="psAv",
                                       tag="psAv")
                    for kc in range(KC):
                        nc.tensor.matmul(
                            ps, xts[kc][:, ts(j2, 128)],
                            w_sb[kc][:, 2 * inner:3 * inner],
                            start=(kc == 0), stop=(kc == KC - 1))
                    nc.vector.tensor_copy(
                        v_sb[it].rearrange(
                            "p (h c) -> p h c", c=dh + 1)[:, :, 0:dh],
                        ps.rearrange("p (h c) -> p h c", c=dh))
                    weave()
                for _ in tq:          # drain leftover transposes
                    pass
                xts = xts_next
            state["chunk_done"] = NB - 1

        # ---- phase B/C emission (single generator; yields the A-chunk
        #      index the NEXT step needs before emitting it) ----
        wout_sb = []                  # filled after phase A pools release
        ysb_open = {}
        pend = []                     # (po, pexp, jt, h, ibx)
        proj_due = []
        qst_tiles = {}
        seq = [(bx, hh) for bx in range(n_ibx) for hh in range(hpc)]

        def qst_req(bx):
            return ((bx + 1) * ib - 1) // 512

        def load_qst(i):
            if i < len(seq) and i not in qst_tiles:
                bx, hh = seq[i]
                if qst_req(bx) > state["chunk_done"]:
                    return
                s2, r2 = divmod(hh * dh, 128)
                t = qst_pool.tile([128, ib], MD, name="qst")
                nc.sync.dma_start(
                    t[r2:r2 + dh, :], qt_dram[s2, r2:r2 + dh, ts(bx, ib)])
                qst_tiles[i] = t

        def emit_tail(po_c, h, ibx, c):
            # normalize rows 0..dh-1 of one half-block by its denominator
            s_, r_ = divmod(h * dh, 128)
            recip_f = tail_pool.tile([1, 512], F32, name="recip_f")
            nc.vector.reciprocal(recip_f, po_c[dh:dh + 1, :])
            bc = tail_pool.tile([dh, 512], F32, name="bc")
            if bcast == "gpsimd":
                nc.gpsimd.partition_broadcast(bc, recip_f)
            else:
                recip = tail_pool.tile([1, 512], MD, name="recip")
                nc.vector.tensor_copy(recip, recip_f)
                pb = psB_pool.tile([dh, 512], F32, name="pb")
                nc.tensor.matmul(pb, ones_sb, recip, start=True, stop=True)
                nc.vector.tensor_copy(bc, pb)
            off = ibx * ib + c * 512
            nc.vector.tensor_mul(
                oT[s_][r_:r_ + dh, off:off + 512], po_c[0:dh, :], bc)

        def emit_proj_group(it, c):
            # one PSUM-group slice of the projection for i-tile `it`
            if c == 0:
                ysb_open[it] = y_pool.tile([128, dim], F32, name="ysb")
            ysb = ysb_open[it]
            ps = psC_pool.tile([128, fc], F32, name="psC")
            for t in range(S):
                nc.tensor.matmul(
                    ps, oT[t][:, ts(it, 128)], wout_sb[t][:, ts(c, fc)],
                    start=(t == 0), stop=(t == S - 1))
            nc.vector.tensor_copy(ysb[:, ts(c, fc)], ps)
            if c == dim // fc - 1:
                nc.sync.dma_start(y[ts(it, 128), :], ysb)
                del ysb_open[it]

        def pop_pend():
            po, pexp, jt, h, ibx = pend.pop(0)
            vcol = slice(h * (dh + 1), (h + 1) * (dh + 1))
            for c in range(ib // 512):
                nc.tensor.matmul(
                    po[c], v_sb[jt][:, vcol], pexp[:, ts(c, 512)],
                    start=(jt == 0), stop=(jt == JT - 1))
            if jt == JT - 1:
                for c in range(ib // 512):
                    emit_tail(po[c], h, ibx, c)

        def b_emit():
            nonlocal proj_due
            for ibx in range(n_ibx):
                for h in range(hpc):
                    gi = ibx * hpc + h
                    yield qst_req(ibx)
                    load_qst(gi)
                    load_qst(gi + 1)
                    s_, r_ = divmod(h * dh, 128)
                    kTh = kT[s_][r_:r_ + dh, :]
                    qTh = qst_tiles.pop(gi)[r_:r_ + dh, :]
                    po = [psO_pool.tile([dh + 1, 512], F32, name="po")
                          for _ in range(ib // 512)]
                    spread = max(1, JT // max(1, -(-len(proj_due) // hpc) + 1))
                    for jt in range(JT):
                        if jt:
                            yield max(qst_req(ibx), jt // 4)
                        psS = psS_pool.tile([128, ib], F32, name="psS")
                        for c in range(ib // 512):
                            nc.tensor.matmul(
                                psS[:, ts(c, 512)], kTh[:, ts(jt, 128)],
                                qTh[:, ts(c, 512)],
                                start=True, stop=True)
                        pexp = pexp_pool.tile([128, ib], MD, name="pexp")
                        nc.scalar.activation(pexp, psS, AF.Exp, scale=scale)
                        pend.append((po, pexp, jt, h, ibx))
                        while len(pend) > 1:
                            pop_pend()
                        if (proj_due and jt % spread == spread - 1
                                and jt < JT - 1):
                            emit_proj_group(*proj_due.pop(0))
                while proj_due:   # leftovers from the previous block
                    emit_proj_group(*proj_due.pop(0))
                proj_due = [(it, c)
                            for it in range(ibx * itpb, (ibx + 1) * itpb)
                            for c in range(dim // fc)]
                if ibx == n_ibx - 1:
                    while pend:
                        pop_pend()
                    for it, c in proj_due:
                        emit_proj_group(it, c)
                    proj_due = []

        # ---- drive phase A, then phase B/C ----
        emit_a()
        actx.close()      # release phase A pools

        # phase B/C pools live in the space freed by phase A
        with (
            tc.tile_pool(name="b_psS", bufs=2, space="PSUM") as psS_pool_,
            tc.tile_pool(name="b_psO", bufs=3, space="PSUM") as psO_pool_,
            tc.tile_pool(name="c_ps", bufs=1, space="PSUM") as psC_pool,
            tc.tile_pool(name="b_pexp", bufs=3) as pexp_pool,
            tc.tile_pool(name="b_qst", bufs=4) as qst_pool,
            tc.tile_pool(name="b_tail", bufs=2) as tail_pool,
            tc.tile_pool(name="c_w", bufs=1) as wout_pool,
            tc.tile_pool(name="c_y", bufs=3) as y_pool,
        ):
            psS_pool, psO_pool = psS_pool_, psO_pool_
            if bcast == "matmul":
                psB_pool = stack.enter_context(
                    tc.tile_pool(name="b_psB", bufs=1, space="PSUM"))
            for s in range(S):
                oT.append(persist.tile([128, n], MD, name="oTs",
                                       tag=f"oT{s}"))
            for t in range(S):
                wo = wout_pool.tile([128, dim], MD, name="wo", tag=f"wo{t}")
                nc.sync.dma_start(wo, wout[ts(t, 128), :])
                wout_sb.append(wo)
            for _ in b_emit():
                pass


_BUILD_CACHE = {}


def build_nc(n=N_FULL, dim=DIM_FULL, hpc=HPC, dh=DH, mm_dt=MM_DT, ib=1024,
             bcast="gpsimd", overlap=True):
    key = (n, dim, hpc, dh, str(mm_dt), ib, bcast, overlap)
    if key in _BUILD_CACHE:
        return _BUILD_CACHE[key]
    inner = hpc * dh
    nc = bacc.Bacc("TRN2", target_bir_lowering=False, debug=False)
    x = nc.dram_tensor("x", [n, dim], F32, kind="ExternalInput").ap()
    wqkv = nc.dram_tensor("w_qkv", [dim, 3 * inner], mm_dt,
                          kind="ExternalInput").ap()
    wout = nc.dram_tensor("w_out", [inner, dim], mm_dt,
                          kind="ExternalInput").ap()
    y = nc.dram_tensor("y", [n, dim], F32, kind="ExternalOutput").ap()
    with tile.TileContext(nc) as tc:
        with nc.allow_low_precision(
                reason="float32r is 4-byte; PSUM accumulation stays fp32"):
            emit_core_kernel(nc, tc, x, wqkv, wout, y, n=n, dim=dim, hpc=hpc,
                             dh=dh, mm_dt=mm_dt, ib=ib, bcast=bcast,
                             overlap=overlap)
    nc.compile()
    _BUILD_CACHE[key] = nc
    return nc


def make_in_maps(x, w_qkv, w_out):
    """Shard full inputs into the 8 per-core input maps."""
    x = np.asarray(x, dtype=np.float32)
    w_qkv = np.asarray(w_qkv, dtype=np.float32)
    w_out = np.asarray(w_out, dtype=np.float32)
    qk_off = HEADS_FULL * DH          # 1024: start of K block in w_qkv
    in_maps = []
    for c in range(N_CORES):
        b, g = divmod(c, GROUPS)
        cols = ts(g, INNER_PC)
        wq = w_qkv[:, cols]
        wk = w_qkv[:, qk_off + g * INNER_PC: qk_off + (g + 1) * INNER_PC]
        wv = w_qkv[:, 2 * qk_off + g * INNER_PC: 2 * qk_off + (g + 1) * INNER_PC]
        in_maps.append({
            "x": np.ascontiguousarray(x[b]),
            "w_qkv": np.ascontiguousarray(np.concatenate([wq, wk, wv], axis=1)),
            "w_out": np.ascontiguousarray(w_out[cols, :]),
        })
    return in_maps


def kernel(x, w_qkv, w_out, b_out, trace=False):
    b_out = np.asarray(b_out, dtype=np.float32)
    nc = build_nc()
    in_maps = make_in_maps(x, w_qkv, w_out)
    res = bass_utils.run_bass_kernel_spmd(
        nc, in_maps, core_ids=list(range(N_CORES)), trace=trace)
    ys = [r["y"] for r in res.results]
    out = np.empty((B_FULL, N_FULL, DIM_FULL), dtype=np.float32)
    for b in range(B_FULL):
        out[b] = ys[GROUPS * b] + ys[GROUPS * b + 1] + b_out[None, :]
    if trace:
        kernel.last_result = res
    return out


# revision 69
# speedup vs baseline: 17149.7840x; 1.0611x over previous
"""Multi-head attention (b=4, n=2048, dim=1024, 16 heads x 64) on 8 Trainium2
NeuronCores.

Sharding: data-parallel over batch (4) x tensor-parallel over head-groups (2).
Each core gets one batch element and 8 heads: it computes its slice of the QKV
projection, full attention for its heads, and a partial output projection.
The host sums the two head-group partials per batch element and adds b_out.

Per-core pipeline (fp32 data; matmul-feeding tiles float32r):
  A:  per 512-wide n-chunk: PE-transpose x tiles into xT chunk tiles (SBUF),
      then qT = Wq^T x^T (staged to DRAM, streamed back in B), kT = Wk^T x^T
      (SBUF-resident, [inner, n] in 128-row strips) and v = x Wv (natural
      [n, inner], augmented with a ones column per head so the PV matmul also
      emits the softmax denominator).
  B:  i-blocks (ib) outer, heads inner: S^T j-tiles = matmul(lhsT=k^T_h
      j-block, rhs=q^T_h i-block) ([j, i] scores); exp on ScalarE
      (1/sqrt(dh) folded into the activation scale); PV matmul accumulates
      O_aug^T = v_aug^T @ P^T in PSUM half-blocks ([dh+1, 512]; last row =
      denominator). Tail: reciprocal of the denominator row, broadcast across
      partitions on GPSIMD, multiply -> normalized O^T strip. The PV matmuls
      trail S/exp by one step in a FIFO that carries across head boundaries,
      so ScalarE (the phase-B bottleneck) never runs dry.
  C:  y = O @ w_out via lhsT = O^T strips, emitted in single-PSUM-group
      slices woven through the NEXT i-block's heads (fills PE slack).
  A/B overlap: emission of B is interleaved into A's chunk emission as soon
  as the chunks a step needs are complete, so ScalarE starts exp work while
  the PE is still on the QKV projection.
"""

from contextlib import ExitStack

import numpy as np

import concourse.mybir as mybir
import concourse.tile as tile
from concourse import bacc, bass_utils

F32 = mybir.dt.float32
AF = mybir.ActivationFunctionType

# Full-problem constants (hardcoded per the harness contract).
B_FULL, N_FULL, DIM_FULL = 4, 2048, 1024
HEADS_FULL, DH = 16, 64
N_CORES = 8
GROUPS = 2                       # head-group (tensor-parallel) factor
HPC = HEADS_FULL // GROUPS       # heads per core = 8
INNER_PC = HPC * DH              # per-core inner dim = 512

# Matmul compute dtype: float32r streams 1 row/cycle (vs 4 for float32) at
# slightly reduced precision. All tiles feeding matmuls carry this dtype
# (producers round into it); numpy float32 maps onto it unchanged.
MM_DT = mybir.dt.float32r


def ts(i, size):
    return slice(i * size, (i + 1) * size)


def emit_core_kernel(nc, tc, x, wqkv, wout, y, *, n, dim, hpc, dh,
                     mm_dt=MM_DT, ib=1024, bcast="gpsimd", overlap=True):
    inner = hpc * dh
    KC = dim // 128          # contraction chunks for the qkv projection
    S = inner // 128         # 128-row strips of the per-core inner dim
    JT = n // 128            # key/value j-tiles
    NB = n // 512            # 512-wide n-chunks in phase A
    ib = min(ib, n)
    assert n % 512 == 0 and dim % 128 == 0 and inner % 128 == 0
    assert ib % 512 == 0 and n % ib == 0
    scale = float(1.0 / np.sqrt(dh))
    MD = mm_dt
    fc = min(512, dim)
    n_ibx = n // ib
    itpb = ib // 128                 # i-tiles per i-block
    state = {"chunk_done": -1}

    stack = ExitStack()
    with stack:
        const_pool = stack.enter_context(tc.tile_pool(name="const", bufs=1))
        persist = stack.enter_context(tc.tile_pool(name="persist", bufs=1))
        dram_pool = stack.enter_context(
            tc.tile_pool(name="dram", bufs=1, space="DRAM"))

        qt_dram = dram_pool.tile([S, 128, n], MD, name="qt_dram")

        # Constants are embedded in the NEFF and DMA'd in (no gpsimd on the
        # startup critical path). Anything that feeds a matmul is rounded
        # into mm_dt via DVE copies.
        ident = const_pool.tile([128, 128], F32, name="ident")
        nc.sync.dma_start(
            ident, nc.inline_tensor(np.eye(128, dtype=np.float32),
                             name=f'identc{nc.next_id()}').ap())
        oneshc = const_pool.tile([128, hpc], F32, name="oneshc")
        nc.sync.dma_start(
            oneshc, nc.inline_tensor(np.ones((128, hpc), np.float32),
                             name=f'onesc{nc.next_id()}').ap())
        if bcast == "matmul":
            ones_f32 = const_pool.tile([1, dh], F32, name="ones_f32")
            nc.gpsimd.memset(ones_f32, 1.0)
            ones_sb = const_pool.tile([1, dh], MD, name="ones_sb")
            nc.vector.tensor_copy(ones_sb, ones_f32)

        # Persistent SBUF tensors: kT strips, v_aug tiles, oT strip 0 (the
        # strip the first two heads write during the A/B overlap; strips 1+
        # are allocated after phase A's pools release).
        kT = []
        for s in range(S):
            kT.append(persist.tile([128, n], MD, name="kTs", tag=f"kT{s}"))
        v_sb = []
        for jt in range(JT):
            vt = persist.tile([128, hpc * (dh + 1)], MD, name="vts",
                              tag=f"v{jt}")
            v_sb.append(vt)
            nc.vector.tensor_copy(
                vt.rearrange("p (h c) -> p h c", c=dh + 1)[:, :, dh:dh + 1],
                oneshc.rearrange("p (h c) -> p h c", c=1))
        oT = []                      # strips allocated after phase A

        # q-block stream pool + loader live below the phase A pools so the
        # first q slices can prefetch while A is still running.
        qst_pool = stack.enter_context(tc.tile_pool(name="b_qst", bufs=2))
        qst_tiles = {}
        seq = [(bx, hh) for bx in range(n_ibx) for hh in range(hpc)]

        def qst_req(bx):
            return ((bx + 1) * ib - 1) // 512

        def load_qst(i):
            if i < len(seq) and i not in qst_tiles:
                bx, hh = seq[i]
                if qst_req(bx) > state["chunk_done"]:
                    return
                s2, r2 = divmod(hh * dh, 128)
                t = qst_pool.tile([128, ib], MD, name="qst")
                nc.sync.dma_start(
                    t[r2:r2 + dh, :], qt_dram[s2, r2:r2 + dh, ts(bx, ib)])
                qst_tiles[i] = t

        # ---- phase A pools ----
        actx = ExitStack()
        w_pool = actx.enter_context(tc.tile_pool(name="a_w", bufs=1))
        xin_pool = actx.enter_context(tc.tile_pool(name="a_xin", bufs=4))
        xts_pool = actx.enter_context(tc.tile_pool(name="a_xts", bufs=2))
        qstage_pool = actx.enter_context(
            tc.tile_pool(name="a_qstage", bufs=2))
        psT_pool = actx.enter_context(
            tc.tile_pool(name="a_psT", bufs=2, space="PSUM"))
        psA_pool = actx.enter_context(
            tc.tile_pool(name="a_ps", bufs=3, space="PSUM"))

        def emit_a():
            # First x tiles are on the critical path; their DMAs go first.
            # The very first tile arrives in column quarters so the first
            # transposes can start before the whole tile lands.
            first_x = []
            for j2 in range(4):
                x_in = xin_pool.tile([128, dim], F32, name="x_in")
                if j2 == 0:
                    for q in range(4):
                        nc.sync.dma_start(x_in[:, ts(q, dim // 4)],
                                          x[ts(j2, 128), ts(q, dim // 4)])
                else:
                    nc.sync.dma_start(x_in, x[ts(j2, 128), :])
                first_x.append(x_in)
            w_sb = []
            for kc in range(KC):
                wt = w_pool.tile([128, 3 * inner], MD, name="wt",
                                 tag=f"w{kc}")
                nc.sync.dma_start(wt, wqkv[ts(kc, 128), :])
                w_sb.append(wt)

            def transpose_ops(nb, xts):
                # lazily emitted (x_in DMA, transpose, copy) for one chunk
                for j2 in range(4):
                    it = nb * 4 + j2
                    if nb == 0:
                        x_in = first_x[j2]
                    else:
                        x_in = xin_pool.tile([128, dim], F32, name="x_in")
                        nc.sync.dma_start(x_in, x[ts(it, 128), :])
                    for kc in range(KC):
                        pt = psT_pool.tile([128, 128], F32, name="pt")
                        nc.tensor.transpose(pt, x_in[:, ts(kc, 128)], ident)
                        nc.vector.tensor_copy(xts[kc][:, ts(j2, 128)], pt)
                        yield None

            def alloc_xts():
                return [xts_pool.tile([128, 512], MD, name="xts",
                                      tag=f"xts{kc}") for kc in range(KC)]

            # transposes run one chunk ahead, woven between matmul groups
            # so the PE is never paced by the DVE copies draining PSUM.
            xts = alloc_xts()
            for _ in transpose_ops(0, xts):
                pass
            for nb in range(NB):
                if nb + 1 < NB:
                    xts_next = alloc_xts()
                    tq = transpose_ops(nb + 1, xts_next)
                else:
                    xts_next, tq = None, iter(())

                def weave(k=3):
                    for _ in range(k):
                        next(tq, None)

                # qT (to DRAM) / kT (SBUF) strips over this 512-wide chunk
                for which in (0, 1):
                    for s in range(S):
                        ps = psA_pool.tile([128, 512], F32, name="psA")
                        base = which * inner + s * 128
                        for kc in range(KC):
                            nc.tensor.matmul(
                                ps, w_sb[kc][:, base:base + 128], xts[kc],
                                start=(kc == 0), stop=(kc == KC - 1))
                        if which == 1:
                            nc.vector.tensor_copy(kT[s][:, ts(nb, 512)], ps)
                        else:
                            qs = qstage_pool.tile([128, 512], MD, name="qs")
                            nc.vector.tensor_copy(qs, ps)
                            nc.sync.dma_start(qt_dram[s, :, ts(nb, 512)], qs)
                        weave()
                # v natural: 4 row-tiles of 128 within this chunk
                for j2 in range(4):
                    it = nb * 4 + j2
                    ps = psA_pool.tile([128, inner], F32, name="psAv",
                                       tag="psAv")
                    for kc in range(KC):
                        nc.tensor.matmul(
                            ps, xts[kc][:, ts(j2, 128)],
                            w_sb[kc][:, 2 * inner:3 * inner],
                            start=(kc == 0), stop=(kc == KC - 1))
                    nc.vector.tensor_copy(
                        v_sb[it].rearrange(
                            "p (h c) -> p h c", c=dh + 1)[:, :, 0:dh],
                        ps.rearrange("p (h c) -> p h c", c=dh))
                    weave()
                for _ in tq:          # drain leftover transposes
                    pass
                xts = xts_next
                state["chunk_done"] = nb
                # prefetch the first q i-block slices as soon as their
                # chunks are staged, so phase B starts without a DMA wait
                if nb == qst_req(seq[0][0]):
                    load_qst(0)
                    load_qst(1)

        # ---- phase B/C emission (single generator; yields the A-chunk
        #      index the NEXT step needs before emitting it) ----
        wout_sb = []                  # filled after phase A pools release
        ysb_open = {}
        pend = []                     # (po, pexp, jt, h, ibx)
        proj_due = []

        def emit_tail(po_c, h, ibx, c):
            # normalize rows 0..dh-1 of one half-block by its denominator
            s_, r_ = divmod(h * dh, 128)
            recip_f = tail_pool.tile([1, 512], F32, name="recip_f")
            nc.vector.reciprocal(recip_f, po_c[dh:dh + 1, :])
            bc = tail_pool.tile([dh, 512], F32, name="bc")
            if bcast == "gpsimd":
                nc.gpsimd.partition_broadcast(bc, recip_f)
            else:
                recip = tail_pool.tile([1, 512], MD, name="recip")
                nc.vector.tensor_copy(recip, recip_f)
                pb = psB_pool.tile([dh, 512], F32, name="pb")
                nc.tensor.matmul(pb, ones_sb, recip, start=True, stop=True)
                nc.vector.tensor_copy(bc, pb)
            off = ibx * ib + c * 512
            nc.vector.tensor_mul(
                oT[s_][r_:r_ + dh, off:off + 512], po_c[0:dh, :], bc)

        def emit_proj_group(it, c):
            # one PSUM-group slice of the projection for i-tile `it`
            if c == 0:
                ysb_open[it] = y_pool.tile([128, dim], F32, name="ysb")
            ysb = ysb_open[it]
            ps = psC_pool.tile([128, fc], F32, name="psC")
            for t in range(S):
                nc.tensor.matmul(
                    ps, oT[t][:, ts(it, 128)], wout_sb[t][:, ts(c, fc)],
                    start=(t == 0), stop=(t == S - 1))
            nc.vector.tensor_copy(ysb[:, ts(c, fc)], ps)
            if c == dim // fc - 1:
                nc.sync.dma_start(y[ts(it, 128), :], ysb)
                del ysb_open[it]

        def pop_pend():
            po, pexp, jt, h, ibx = pend.pop(0)
            vcol = slice(h * (dh + 1), (h + 1) * (dh + 1))
            for c in range(ib // 512):
                nc.tensor.matmul(
                    po[c], v_sb[jt][:, vcol], pexp[:, ts(c, 512)],
                    start=(jt == 0), stop=(jt == JT - 1))
            if jt == JT - 1:
                for c in range(ib // 512):
                    emit_tail(po[c], h, ibx, c)

        head_state = {}               # gi -> (qTh, kTh, po)

        def emit_s(k, steps):
            # S matmuls for flat step k (allocates the head's tiles on its
            # first step; runs one step AHEAD of exp so ScalarE never waits
            # on the PE at head boundaries)
            ibx, h, jt = steps[k]
            gi = ibx * hpc + h
            if jt == 0:
                load_qst(gi)
                load_qst(gi + 1)
                s_, r_ = divmod(h * dh, 128)
                head_state[gi] = (
                    qst_tiles.pop(gi)[r_:r_ + dh, :],
                    kT[s_][r_:r_ + dh, :],
                    [psO_pool.tile([dh + 1, 512], F32, name="po")
                     for _ in range(ib // 512)])
            qTh, kTh, po = head_state[gi]
            psS = psS_pool.tile([128, ib], F32, name="psS")
            for c in range(ib // 512):
                nc.tensor.matmul(
                    psS[:, ts(c, 512)], kTh[:, ts(jt, 128)],
                    qTh[:, ts(c, 512)], start=True, stop=True)
            return psS, po

        def b_emit():
            nonlocal proj_due
            steps = [(bx, hh, jt) for bx in range(n_ibx)
                     for hh in range(hpc) for jt in range(JT)]
            s_ahead = None            # (psS, po) for step k, S already done
            for k, (ibx, h, jt) in enumerate(steps):
                yield max(qst_req(ibx), jt // 4)
                if s_ahead is None:
                    s_ahead = emit_s(k, steps)
                psS, po = s_ahead
                # S for step k+1 goes out before exp(k)
                nx = k + 1
                if nx < len(steps) and (steps[nx][2] // 4 <=
                                        state["chunk_done"]) and \
                        qst_req(steps[nx][0]) <= state["chunk_done"]:
                    s_ahead = emit_s(nx, steps)
                else:
                    s_ahead = None
                pexp = pexp_pool.tile([128, ib], MD, name="pexp")
                nc.scalar.activation(pexp, psS, AF.Exp, scale=scale)
                pend.append((po, pexp, jt, h, ibx))
                while len(pend) > 1:
                    pop_pend()
                if jt == JT - 1:
                    head_state.pop(ibx * hpc + h, None)
                if jt == 0:
                    spread_n = -(-len(proj_due) // hpc) + 1
                if (proj_due and jt % max(1, JT // max(1, spread_n)) ==
                        JT // max(1, spread_n) - 1 and jt < JT - 1):
                    emit_proj_group(*proj_due.pop(0))
                if jt == JT - 1 and h == hpc - 1:
                    # end of an i-block
                    while proj_due:
                        emit_proj_group(*proj_due.pop(0))
                    proj_due = [(it, c)
                                for it in range(ibx * itpb, (ibx + 1) * itpb)
                                for c in range(dim // fc)]
                    if ibx == n_ibx - 1:
                        while pend:
                            pop_pend()
                        for it, c in proj_due:
                            emit_proj_group(it, c)
                        proj_due = []

        # ---- drive phase A, then phase B/C ----
        emit_a()
        actx.close()      # release phase A pools

        # phase B/C pools live in the space freed by phase A
        with (
            tc.tile_pool(name="b_psS", bufs=2, space="PSUM") as psS_pool_,
            tc.tile_pool(name="b_psO", bufs=3, space="PSUM") as psO_pool_,
            tc.tile_pool(name="c_ps", bufs=1, space="PSUM") as psC_pool,
            tc.tile_pool(name="b_pexp", bufs=3) as pexp_pool,
            tc.tile_pool(name="b_tail", bufs=2) as tail_pool,
            tc.tile_pool(name="c_w", bufs=1) as wout_pool,
            tc.tile_pool(name="c_y", bufs=3) as y_pool,
        ):
            psS_pool, psO_pool = psS_pool_, psO_pool_
            if bcast == "matmul":
                psB_pool = stack.enter_context(
                    tc.tile_pool(name="b_psB", bufs=1, space="PSUM"))
            for s in range(S):
                oT.append(persist.tile([128, n], MD, name="oTs",
                                       tag=f"oT{s}"))
            for t in range(S):
                wo = wout_pool.tile([128, dim], MD, name="wo", tag=f"wo{t}")
                nc.sync.dma_start(wo, wout[ts(t, 128), :])
                wout_sb.append(wo)
            for _ in b_emit():
                pass


_BUILD_CACHE = {}


def build_nc(n=N_FULL, dim=DIM_FULL, hpc=HPC, dh=DH, mm_dt=MM_DT, ib=1024,
             bcast="gpsimd", overlap=True, reps=1):
    key = (n, dim, hpc, dh, str(mm_dt), ib, bcast, overlap, reps)
    if key in _BUILD_CACHE:
        return _BUILD_CACHE[key]
    inner = hpc * dh
    nc = bacc.Bacc("TRN2", target_bir_lowering=False, debug=False)
    x = nc.dram_tensor("x", [n, dim], F32, kind="ExternalInput").ap()
    wqkv = nc.dram_tensor("w_qkv", [dim, 3 * inner], mm_dt,
                          kind="ExternalInput").ap()
    wout = nc.dram_tensor("w_out", [inner, dim], mm_dt,
                          kind="ExternalInput").ap()
    y = nc.dram_tensor("y", [n, dim], F32, kind="ExternalOutput").ap()
    with tile.TileContext(nc) as tc:
        with nc.allow_low_precision(
                reason="float32r is 4-byte; PSUM accumulation stays fp32"):
            for _ in range(reps):
                emit_core_kernel(nc, tc, x, wqkv, wout, y, n=n, dim=dim,
                                 hpc=hpc, dh=dh, mm_dt=mm_dt, ib=ib,
                                 bcast=bcast, overlap=overlap)
    nc.compile()
    _BUILD_CACHE[key] = nc
    return nc


def make_in_maps(x, w_qkv, w_out):
    """Shard full inputs into the 8 per-core input maps."""
    x = np.asarray(x, dtype=np.float32)
    w_qkv = np.asarray(w_qkv, dtype=np.float32)
    w_out = np.asarray(w_out, dtype=np.float32)
    qk_off = HEADS_FULL * DH          # 1024: start of K block in w_qkv
    in_maps = []
    for c in range(N_CORES):
        b, g = divmod(c, GROUPS)
        cols = ts(g, INNER_PC)
        wq = w_qkv[:, cols]
        wk = w_qkv[:, qk_off + g * INNER_PC: qk_off + (g + 1) * INNER_PC]
        wv = w_qkv[:, 2 * qk_off + g * INNER_PC: 2 * qk_off + (g + 1) * INNER_PC]
        in_maps.append({
            "x": np.ascontiguousarray(x[b]),
            "w_qkv": np.ascontiguousarray(np.concatenate([wq, wk, wv], axis=1)),
            "w_out": np.ascontiguousarray(w_out[cols, :]),
        })
    return in_maps


def kernel(x, w_qkv, w_out, b_out, trace=False):
    b_out = np.asarray(b_out, dtype=np.float32)
    nc = build_nc()
    in_maps = make_in_maps(x, w_qkv, w_out)
    res = bass_utils.run_bass_kernel_spmd(
        nc, in_maps, core_ids=list(range(N_CORES)), trace=trace)
    ys = [r["y"] for r in res.results]
    out = np.empty((B_FULL, N_FULL, DIM_FULL), dtype=np.float32)
    for b in range(B_FULL):
        out[b] = ys[GROUPS * b] + ys[GROUPS * b + 1] + b_out[None, :]
    if trace:
        kernel.last_result = res
    return out


# revision 76
# speedup vs baseline: 17933.2510x; 1.0457x over previous
"""Multi-head attention (b=4, n=2048, dim=1024, 16 heads x 64) on 8 Trainium2
NeuronCores.

Sharding: data-parallel over batch (4) x tensor-parallel over head-groups (2).
Each core gets one batch element and 8 heads: it computes its slice of the QKV
projection, full attention for its heads, and a partial output projection.
The host sums the two head-group partials per batch element and adds b_out.

Per-core pipeline (fp32 data; matmul-feeding tiles float32r):
  A:  per 512-wide n-chunk: PE-transpose x tiles into xT chunk tiles (SBUF),
      then qT = Wq^T x^T (staged to DRAM, streamed back in B), kT = Wk^T x^T
      (SBUF-resident, [inner, n] in 128-row strips) and v = x Wv (natural
      [n, inner], augmented with a ones column per head so the PV matmul also
      emits the softmax denominator).
  B:  i-blocks (ib) outer, heads inner: S^T j-tiles = matmul(lhsT=k^T_h
      j-block, rhs=q^T_h i-block) ([j, i] scores); exp on ScalarE
      (1/sqrt(dh) folded into the activation scale); PV matmul accumulates
      O_aug^T = v_aug^T @ P^T in PSUM half-blocks ([dh+1, 512]; last row =
      denominator). Tail: reciprocal of the denominator row, broadcast across
      partitions on GPSIMD, multiply -> normalized O^T strip. The PV matmuls
      trail S/exp by one step in a FIFO that carries across head boundaries,
      so ScalarE (the phase-B bottleneck) never runs dry.
  C:  y = O @ w_out via lhsT = O^T strips, emitted in single-PSUM-group
      slices woven through the NEXT i-block's heads (fills PE slack).
  A/B overlap: emission of B is interleaved into A's chunk emission as soon
  as the chunks a step needs are complete, so ScalarE starts exp work while
  the PE is still on the QKV projection.
"""

from contextlib import ExitStack

import numpy as np

import concourse.mybir as mybir
import concourse.tile as tile
from concourse import bacc, bass_utils

F32 = mybir.dt.float32
AF = mybir.ActivationFunctionType

# Full-problem constants (hardcoded per the harness contract).
B_FULL, N_FULL, DIM_FULL = 4, 2048, 1024
HEADS_FULL, DH = 16, 64
N_CORES = 8
GROUPS = 2                       # head-group (tensor-parallel) factor
HPC = HEADS_FULL // GROUPS       # heads per core = 8
INNER_PC = HPC * DH              # per-core inner dim = 512

# Matmul compute dtype: float32r streams 1 row/cycle (vs 4 for float32) at
# slightly reduced precision. All tiles feeding matmuls carry this dtype
# (producers round into it); numpy float32 maps onto it unchanged.
MM_DT = mybir.dt.float32r


def ts(i, size):
    return slice(i * size, (i + 1) * size)


def emit_core_kernel(nc, tc, x, wqkv, wout, y, *, n, dim, hpc, dh,
                     mm_dt=MM_DT, ib=1024, bcast="gpsimd", overlap=True,
                     s_lead=True):
    inner = hpc * dh
    KC = dim // 128          # contraction chunks for the qkv projection
    S = inner // 128         # 128-row strips of the per-core inner dim
    JT = n // 128            # key/value j-tiles
    NB = n // 512            # 512-wide n-chunks in phase A
    ib = min(ib, n)
    assert n % 512 == 0 and dim % 128 == 0 and inner % 128 == 0
    assert ib % 512 == 0 and n % ib == 0
    scale = float(1.0 / np.sqrt(dh))
    MD = mm_dt
    fc = min(512, dim)
    n_ibx = n // ib
    itpb = ib // 128                 # i-tiles per i-block
    state = {"chunk_done": -1}

    stack = ExitStack()
    with stack:
        const_pool = stack.enter_context(tc.tile_pool(name="const", bufs=1))
        persist = stack.enter_context(tc.tile_pool(name="persist", bufs=1))
        dram_pool = stack.enter_context(
            tc.tile_pool(name="dram", bufs=1, space="DRAM"))

        qt_dram = dram_pool.tile([S, 128, n], MD, name="qt_dram")

        # Constants are embedded in the NEFF and DMA'd in (no gpsimd on the
        # startup critical path). Anything that feeds a matmul is rounded
        # into mm_dt via DVE copies.
        ident = const_pool.tile([128, 128], F32, name="ident")
        nc.sync.dma_start(
            ident, nc.inline_tensor(np.eye(128, dtype=np.float32),
                             name=f'identc{nc.next_id()}').ap())
        oneshc = const_pool.tile([128, hpc], F32, name="oneshc")
        nc.sync.dma_start(
            oneshc, nc.inline_tensor(np.ones((128, hpc), np.float32),
                             name=f'onesc{nc.next_id()}').ap())
        if bcast == "matmul":
            ones_f32 = const_pool.tile([1, dh], F32, name="ones_f32")
            nc.gpsimd.memset(ones_f32, 1.0)
            ones_sb = const_pool.tile([1, dh], MD, name="ones_sb")
            nc.vector.tensor_copy(ones_sb, ones_f32)

        # Persistent SBUF tensors: kT strips, v_aug tiles, oT strip 0 (the
        # strip the first two heads write during the A/B overlap; strips 1+
        # are allocated after phase A's pools release).
        kT = []
        for s in range(S):
            kT.append(persist.tile([128, n], MD, name="kTs", tag=f"kT{s}"))
        v_sb = []
        for jt in range(JT):
            vt = persist.tile([128, hpc * (dh + 1)], MD, name="vts",
                              tag=f"v{jt}")
            v_sb.append(vt)
            nc.vector.tensor_copy(
                vt.rearrange("p (h c) -> p h c", c=dh + 1)[:, :, dh:dh + 1],
                oneshc.rearrange("p (h c) -> p h c", c=1))
        oT = []                      # strips allocated after phase A

        # q-block stream pool + loader live below the phase A pools so the
        # first q slices can prefetch while A is still running.
        qst_pool = stack.enter_context(tc.tile_pool(name="b_qst", bufs=2))
        qst_tiles = {}
        seq = [(bx, hh) for bx in range(n_ibx) for hh in range(hpc)]

        def qst_req(bx):
            return ((bx + 1) * ib - 1) // 512

        def load_qst(i):
            if i < len(seq) and i not in qst_tiles:
                bx, hh = seq[i]
                if qst_req(bx) > state["chunk_done"]:
                    return
                s2, r2 = divmod(hh * dh, 128)
                t = qst_pool.tile([128, ib], MD, name="qst")
                nc.sync.dma_start(
                    t[r2:r2 + dh, :], qt_dram[s2, r2:r2 + dh, ts(bx, ib)])
                qst_tiles[i] = t

        # ---- phase A pools ----
        actx = ExitStack()
        w_pool = actx.enter_context(tc.tile_pool(name="a_w", bufs=1))
        xin_pool = actx.enter_context(tc.tile_pool(name="a_xin", bufs=4))
        xts_pool = actx.enter_context(tc.tile_pool(name="a_xts", bufs=2))
        qstage_pool = actx.enter_context(
            tc.tile_pool(name="a_qstage", bufs=2))
        psT_pool = actx.enter_context(
            tc.tile_pool(name="a_psT", bufs=2, space="PSUM"))
        psA_pool = actx.enter_context(
            tc.tile_pool(name="a_ps", bufs=3, space="PSUM"))

        def emit_a():
            # First x tiles are on the critical path; their DMAs go first.
            # The very first tile arrives in column quarters so the first
            # transposes can start before the whole tile lands.
            first_x = []
            for j2 in range(4):
                x_in = xin_pool.tile([128, dim], F32, name="x_in")
                if j2 == 0:
                    for q in range(4):
                        nc.sync.dma_start(x_in[:, ts(q, dim // 4)],
                                          x[ts(j2, 128), ts(q, dim // 4)])
                else:
                    nc.sync.dma_start(x_in, x[ts(j2, 128), :])
                first_x.append(x_in)
            w_sb = []
            for kc in range(KC):
                wt = w_pool.tile([128, 3 * inner], MD, name="wt",
                                 tag=f"w{kc}")
                nc.sync.dma_start(wt, wqkv[ts(kc, 128), :])
                w_sb.append(wt)

            def transpose_ops(nb, xts):
                # lazily emitted (x_in DMA, transpose, copy) for one chunk
                for j2 in range(4):
                    it = nb * 4 + j2
                    if nb == 0:
                        x_in = first_x[j2]
                    else:
                        x_in = xin_pool.tile([128, dim], F32, name="x_in")
                        nc.sync.dma_start(x_in, x[ts(it, 128), :])
                    for kc in range(KC):
                        pt = psT_pool.tile([128, 128], F32, name="pt")
                        nc.tensor.transpose(pt, x_in[:, ts(kc, 128)], ident)
                        nc.vector.tensor_copy(xts[kc][:, ts(j2, 128)], pt)
                        yield None

            def alloc_xts():
                return [xts_pool.tile([128, 512], MD, name="xts",
                                      tag=f"xts{kc}") for kc in range(KC)]

            # transposes run one chunk ahead, woven between matmul groups
            # so the PE is never paced by the DVE copies draining PSUM.
            xts = alloc_xts()
            for _ in transpose_ops(0, xts):
                pass
            for nb in range(NB):
                if nb + 1 < NB:
                    xts_next = alloc_xts()
                    tq = transpose_ops(nb + 1, xts_next)
                else:
                    xts_next, tq = None, iter(())

                def weave(k=3):
                    for _ in range(k):
                        next(tq, None)

                # qT (to DRAM) / kT (SBUF) strips over this 512-wide chunk
                for which in (0, 1):
                    for s in range(S):
                        ps = psA_pool.tile([128, 512], F32, name="psA")
                        base = which * inner + s * 128
                        for kc in range(KC):
                            nc.tensor.matmul(
                                ps, w_sb[kc][:, base:base + 128], xts[kc],
                                start=(kc == 0), stop=(kc == KC - 1))
                        if which == 1:
                            nc.vector.tensor_copy(kT[s][:, ts(nb, 512)], ps)
                        else:
                            qs = qstage_pool.tile([128, 512], MD, name="qs")
                            nc.vector.tensor_copy(qs, ps)
                            nc.sync.dma_start(qt_dram[s, :, ts(nb, 512)], qs)
                        weave()
                # v natural: 4 row-tiles of 128 within this chunk
                for j2 in range(4):
                    it = nb * 4 + j2
                    ps = psA_pool.tile([128, inner], F32, name="psAv",
                                       tag="psAv")
                    for kc in range(KC):
                        nc.tensor.matmul(
                            ps, xts[kc][:, ts(j2, 128)],
                            w_sb[kc][:, 2 * inner:3 * inner],
                            start=(kc == 0), stop=(kc == KC - 1))
                    nc.vector.tensor_copy(
                        v_sb[it].rearrange(
                            "p (h c) -> p h c", c=dh + 1)[:, :, 0:dh],
                        ps.rearrange("p (h c) -> p h c", c=dh))
                    weave()
                for _ in tq:          # drain leftover transposes
                    pass
                xts = xts_next
                state["chunk_done"] = nb
                # prefetch the first q i-block slices as soon as their
                # chunks are staged, so phase B starts without a DMA wait
                if nb == qst_req(seq[0][0]):
                    load_qst(0)
                    load_qst(1)

        # ---- phase B/C emission (single generator; yields the A-chunk
        #      index the NEXT step needs before emitting it) ----
        wout_sb = []                  # filled after phase A pools release
        ysb_open = {}
        pend = []                     # (po, pexp, jt, h, ibx)
        proj_due = []

        def emit_tail(po_c, h, ibx, c):
            # normalize rows 0..dh-1 of one half-block by its denominator
            s_, r_ = divmod(h * dh, 128)
            recip_f = tail_pool.tile([1, 512], F32, name="recip_f")
            nc.vector.reciprocal(recip_f, po_c[dh:dh + 1, :])
            bc = tail_pool.tile([dh, 512], F32, name="bc")
            if bcast == "gpsimd":
                nc.gpsimd.partition_broadcast(bc, recip_f)
            else:
                recip = tail_pool.tile([1, 512], MD, name="recip")
                nc.vector.tensor_copy(recip, recip_f)
                pb = psB_pool.tile([dh, 512], F32, name="pb")
                nc.tensor.matmul(pb, ones_sb, recip, start=True, stop=True)
                nc.vector.tensor_copy(bc, pb)
            off = ibx * ib + c * 512
            nc.vector.tensor_mul(
                oT[s_][r_:r_ + dh, off:off + 512], po_c[0:dh, :], bc)

        def emit_proj_group(it, c, final=False):
            # one PSUM-group slice of the projection for i-tile `it`. The
            # final flush borrows the (by then idle) psS slots so its groups
            # pipeline instead of serializing on the single psC bank.
            if c == 0:
                ysb_open[it] = y_pool.tile([128, dim], F32, name="ysb")
            ysb = ysb_open[it]
            if final:
                ps = psS_pool.tile([128, fc], F32, name="psS")
            else:
                ps = psC_pool.tile([128, fc], F32, name="psC")
            for t in range(S):
                nc.tensor.matmul(
                    ps, oT[t][:, ts(it, 128)], wout_sb[t][:, ts(c, fc)],
                    start=(t == 0), stop=(t == S - 1))
            nc.vector.tensor_copy(ysb[:, ts(c, fc)], ps)
            if c == dim // fc - 1:
                nc.sync.dma_start(y[ts(it, 128), :], ysb)
                del ysb_open[it]

        def pop_pend():
            po, pexp, jt, h, ibx = pend.pop(0)
            vcol = slice(h * (dh + 1), (h + 1) * (dh + 1))
            for c in range(ib // 512):
                nc.tensor.matmul(
                    po[c], v_sb[jt][:, vcol], pexp[:, ts(c, 512)],
                    start=(jt == 0), stop=(jt == JT - 1))
            if jt == JT - 1:
                for c in range(ib // 512):
                    emit_tail(po[c], h, ibx, c)

        head_state = {}               # gi -> (qTh, kTh, po)

        def emit_s(k, steps):
            # S matmuls for flat step k (allocates the head's tiles on its
            # first step; runs one step AHEAD of exp so ScalarE never waits
            # on the PE at head boundaries)
            ibx, h, jt = steps[k]
            gi = ibx * hpc + h
            if jt == 0:
                load_qst(gi)
                load_qst(gi + 1)
                s_, r_ = divmod(h * dh, 128)
                head_state[gi] = (
                    qst_tiles.pop(gi)[r_:r_ + dh, :],
                    kT[s_][r_:r_ + dh, :],
                    [psO_pool.tile([dh + 1, 512], F32, name="po")
                     for _ in range(ib // 512)])
            qTh, kTh, po = head_state[gi]
            psS = psS_pool.tile([128, ib], F32, name="psS")
            for c in range(ib // 512):
                nc.tensor.matmul(
                    psS[:, ts(c, 512)], kTh[:, ts(jt, 128)],
                    qTh[:, ts(c, 512)], start=True, stop=True)
            return psS, po

        def b_emit():
            nonlocal proj_due
            steps = [(bx, hh, jt) for bx in range(n_ibx)
                     for hh in range(hpc) for jt in range(JT)]
            s_ahead = None            # (psS, po) for step k, S already done
            for k, (ibx, h, jt) in enumerate(steps):
                yield max(qst_req(ibx), jt // 4)
                if s_ahead is None:
                    s_ahead = emit_s(k, steps)
                psS, po = s_ahead
                nx = k + 1
                if s_lead:
                    # S for step k+1 goes out before exp(k)
                    if nx < len(steps) and (steps[nx][2] // 4 <=
                                            state["chunk_done"]) and \
                            qst_req(steps[nx][0]) <= state["chunk_done"]:
                        s_ahead = emit_s(nx, steps)
                    else:
                        s_ahead = None
                pexp = pexp_pool.tile([128, ib], MD, name="pexp")
                nc.scalar.activation(pexp, psS, AF.Exp, scale=scale)
                if not s_lead:
                    if nx < len(steps) and (steps[nx][2] // 4 <=
                                            state["chunk_done"]) and \
                            qst_req(steps[nx][0]) <= state["chunk_done"]:
                        s_ahead = emit_s(nx, steps)
                    else:
                        s_ahead = None
                pend.append((po, pexp, jt, h, ibx))
                while len(pend) > 1:
                    pop_pend()
                if jt == JT - 1:
                    head_state.pop(ibx * hpc + h, None)
                if jt == 0:
                    spread_n = -(-len(proj_due) // hpc) + 1
                if (proj_due and jt % max(1, JT // max(1, spread_n)) ==
                        JT // max(1, spread_n) - 1 and jt < JT - 1):
                    emit_proj_group(*proj_due.pop(0))
                if jt == JT - 1 and h == hpc - 1:
                    # end of an i-block
                    while proj_due:
                        emit_proj_group(*proj_due.pop(0))
                    proj_due = [(it, c)
                                for it in range(ibx * itpb, (ibx + 1) * itpb)
                                for c in range(dim // fc)]
                    if ibx == n_ibx - 1:
                        while pend:
                            pop_pend()
                        for it, c in proj_due:
                            emit_proj_group(it, c, final=True)
                        proj_due = []

        # ---- drive phase A, then phase B/C ----
        emit_a()
        actx.close()      # release phase A pools

        # phase B/C pools live in the space freed by phase A
        with (
            tc.tile_pool(name="b_psS", bufs=2, space="PSUM") as psS_pool_,
            tc.tile_pool(name="b_psO", bufs=3, space="PSUM") as psO_pool_,
            tc.tile_pool(name="c_ps", bufs=1, space="PSUM") as psC_pool,
            tc.tile_pool(name="b_pexp", bufs=4) as pexp_pool,
            tc.tile_pool(name="b_tail", bufs=3) as tail_pool,
            tc.tile_pool(name="c_w", bufs=1) as wout_pool,
            tc.tile_pool(name="c_y", bufs=3) as y_pool,
        ):
            psS_pool, psO_pool = psS_pool_, psO_pool_
            if bcast == "matmul":
                psB_pool = stack.enter_context(
                    tc.tile_pool(name="b_psB", bufs=1, space="PSUM"))
            for s in range(S):
                oT.append(persist.tile([128, n], MD, name="oTs",
                                       tag=f"oT{s}"))
            for t in range(S):
                wo = wout_pool.tile([128, dim], MD, name="wo", tag=f"wo{t}")
                nc.sync.dma_start(wo, wout[ts(t, 128), :])
                wout_sb.append(wo)
            for _ in b_emit():
                pass


_BUILD_CACHE = {}


def build_nc(n=N_FULL, dim=DIM_FULL, hpc=HPC, dh=DH, mm_dt=MM_DT, ib=1024,
             bcast="gpsimd", overlap=True, reps=1, s_lead=True):
    key = (n, dim, hpc, dh, str(mm_dt), ib, bcast, overlap, reps, s_lead)
    if key in _BUILD_CACHE:
        return _BUILD_CACHE[key]
    inner = hpc * dh
    nc = bacc.Bacc("TRN2", target_bir_lowering=False, debug=False)
    x = nc.dram_tensor("x", [n, dim], F32, kind="ExternalInput").ap()
    wqkv = nc.dram_tensor("w_qkv", [dim, 3 * inner], mm_dt,
                          kind="ExternalInput").ap()
    wout = nc.dram_tensor("w_out", [inner, dim], mm_dt,
                          kind="ExternalInput").ap()
    y = nc.dram_tensor("y", [n, dim], F32, kind="ExternalOutput").ap()
    with tile.TileContext(nc) as tc:
        with nc.allow_low_precision(
                reason="float32r is 4-byte; PSUM accumulation stays fp32"):
            for _ in range(reps):
                emit_core_kernel(nc, tc, x, wqkv, wout, y, n=n, dim=dim,
                                 hpc=hpc, dh=dh, mm_dt=mm_dt, ib=ib,
                                 bcast=bcast, overlap=overlap, s_lead=s_lead)
    nc.compile()
    _BUILD_CACHE[key] = nc
    return nc


def make_in_maps(x, w_qkv, w_out):
    """Shard full inputs into the 8 per-core input maps."""
    x = np.asarray(x, dtype=np.float32)
    w_qkv = np.asarray(w_qkv, dtype=np.float32)
    w_out = np.asarray(w_out, dtype=np.float32)
    qk_off = HEADS_FULL * DH          # 1024: start of K block in w_qkv
    in_maps = []
    for c in range(N_CORES):
        b, g = divmod(c, GROUPS)
        cols = ts(g, INNER_PC)
        wq = w_qkv[:, cols]
        wk = w_qkv[:, qk_off + g * INNER_PC: qk_off + (g + 1) * INNER_PC]
        wv = w_qkv[:, 2 * qk_off + g * INNER_PC: 2 * qk_off + (g + 1) * INNER_PC]
        in_maps.append({
            "x": np.ascontiguousarray(x[b]),
            "w_qkv": np.ascontiguousarray(np.concatenate([wq, wk, wv], axis=1)),
            "w_out": np.ascontiguousarray(w_out[cols, :]),
        })
    return in_maps


def kernel(x, w_qkv, w_out, b_out, trace=False):
    b_out = np.asarray(b_out, dtype=np.float32)
    nc = build_nc()
    in_maps = make_in_maps(x, w_qkv, w_out)
    res = bass_utils.run_bass_kernel_spmd(
        nc, in_maps, core_ids=list(range(N_CORES)), trace=trace)
    ys = [r["y"] for r in res.results]
    out = np.empty((B_FULL, N_FULL, DIM_FULL), dtype=np.float32)
    for b in range(B_FULL):
        out[b] = ys[GROUPS * b] + ys[GROUPS * b + 1] + b_out[None, :]
    if trace:
        kernel.last_result = res
    return out
